# revision 4
# baseline (speedup 1.0000x reference)
"""Trainium2 Bass kernel for nn_Model_1580547969651.

Math (from the reference):
    s    = x @ sum(y, axis=0)          # (B,) row-sums of x @ y^T
    h    = hardswish(s)                # s * clip(s+3, 0, 6) / 6
    out  = clip(h + noise, -0.5, 0.5)  # (B, 1)

Strategy: data-parallel shard x/noise (and y) on batch across 8 cores.
Each core reduces its y shard to a partial ysum (PE matmul with a ones
vector), a 16 KB AllReduce produces the full ysum, which is broadcast
across partitions (K=1 matmul) and dotted against the x shard with the
fused DVE tensor_tensor_reduce. Elementwise tail is trivial.
"""

import numpy as np

from concourse import bass, bacc, mybir, tile
from concourse.bass_utils import run_bass_kernel_spmd

B = 8192
F = 4096
NCORES = 8
BL = B // NCORES        # 1024 rows per core
NBT = BL // 128         # 8 batch tiles of 128 rows per core
NFC = F // 512          # 8 free-dim chunks of 512 (one PSUM bank each)
FP32 = mybir.dt.float32

_CACHE: dict = {}


def _build():
    nc = bacc.Bacc(
        "TRN2",
        target_bir_lowering=False,
        debug=False,
        num_devices=NCORES,
    )

    x_d = nc.dram_tensor("x", [BL, F], FP32, kind="ExternalInput")
    y_d = nc.dram_tensor("y", [BL, F], FP32, kind="ExternalInput")
    nz_d = nc.dram_tensor("noise", [BL, 1], FP32, kind="ExternalInput")
    out_d = nc.dram_tensor("out", [BL, 1], FP32, kind="ExternalOutput")

    y_r = y_d[:, :].rearrange("(k p) f -> k p f", p=128)    # (8, 128, F)
    x_r = x_d[:, :].rearrange("(k p) f -> k p f", p=128)    # (8, 128, F)
    nz_r = nz_d[:, 0].rearrange("(k p) -> p k", p=128)      # (128, 8)
    out_r = out_d[:, 0].rearrange("(k p) -> p k", p=128)    # (128, 8)

    with tile.TileContext(nc) as tc:
        with (
            tc.tile_pool(name="ypool", bufs=3) as ypool,
            tc.tile_pool(name="xpool", bufs=5) as xpool,
            tc.tile_pool(name="small", bufs=1) as small,
            tc.tile_pool(name="scratch", bufs=1) as scratch,
            tc.tile_pool(name="psum", bufs=1, space="PSUM") as psum,
            tc.tile_pool(name="dram", bufs=1, space="DRAM") as dram,
        ):
            ones_col = small.tile([128, 1], FP32)
            nc.gpsimd.memset(ones_col[:], 1.0)
            ones_row = small.tile([1, 128], FP32)
            nc.gpsimd.memset(ones_row[:], 1.0)

            # ---- phase 1: partial ysum = sum over this shard's y rows ----
            # ones_col.T @ y_tile accumulated over the 8 batch tiles, giving
            # a (1, F) row in PSUM (512-wide chunks, one bank each).
            ysum_ps = psum.tile([1, F], FP32, tag="ps")
            xtiles = []
            for k in range(NBT):
                ytile = ypool.tile([128, F], FP32, tag="y")
                nc.sync.dma_start(ytile[:], y_r[k])
                for j in range(NFC):
                    nc.tensor.matmul(
                        ysum_ps[0:1, j * 512:(j + 1) * 512],
                        ones_col[:],
                        ytile[:, j * 512:(j + 1) * 512],
                        start=(k == 0),
                        stop=(k == NBT - 1),
                    )

            # ---- x stream: issued after y on the same HWDGE ring so y
            # (which gates the AllReduce) keeps strict DMA priority ----
            for k in range(NBT):
                xtile = xpool.tile([128, F], FP32, tag="x")
                nc.sync.dma_start(xtile[:], x_r[k])
                xtiles.append(xtile)

            # ---- AllReduce the 16 KB partial ysum across the 8 cores ----
            cc_in = dram.tile([1, F], FP32)
            cc_out = dram.tile([1, F], FP32)
            ysum_part = small.tile([1, F], FP32)
            # PSUM isn't DMA-readable; bounce through SBUF. Split across
            # DVE/ACT so the single-partition copy isn't serialized on one
            # engine (it sits on the critical path before the AllReduce).
            nc.vector.tensor_copy(ysum_part[0:1, :F // 2], ysum_ps[0:1, :F // 2])
            nc.scalar.copy(ysum_part[0:1, F // 2:], ysum_ps[0:1, F // 2:])
            nc.gpsimd.dma_start(cc_in[:], ysum_part[:])
            nc.gpsimd.collective_compute(
                "AllReduce",
                mybir.AluOpType.add,
                replica_groups=[list(range(NCORES))],
                ins=[cc_in.opt()],
                outs=[cc_out.opt()],
            )
            ysum_sb = small.tile([1, F], FP32)
            nc.gpsimd.dma_start(ysum_sb[:], cc_out[:])

            # ---- broadcast ysum across partitions: ones_row.T @ ysum_sb ----
            # (reuses the phase-1 PSUM banks once ysum_ps is drained)
            bc = psum.tile([128, F], FP32, tag="ps")
            for j in range(NFC):
                nc.tensor.matmul(
                    bc[:, j * 512:(j + 1) * 512],
                    ones_row[:],
                    ysum_sb[0:1, j * 512:(j + 1) * 512],
                    start=True,
                    stop=True,
                )

            # ---- phase 2: s[b] = dot(x[b, :], ysum) via fused mult+reduce ----
            s_all = small.tile([128, NBT], FP32)
            for k in range(NBT):
                prod = scratch.tile([128, F], FP32, tag="sc")
                nc.vector.scalar_tensor_tensor(
                    out=prod[:],
                    in0=xtiles[k][:],
                    scalar=1.0,
                    in1=bc[:],
                    op0=mybir.AluOpType.mult,
                    op1=mybir.AluOpType.mult,
                    accum_out=s_all[:, k:k + 1],
                )

            # ---- tail: hardswish, + noise, hardtanh ----
            noise_t = small.tile([128, NBT], FP32)
            nc.gpsimd.dma_start(noise_t[:], nz_r)

            t = small.tile([128, NBT], FP32)
            # t = clip(s + 3, 0, 6)
            nc.vector.tensor_scalar(
                out=t[:], in0=s_all[:], scalar1=3.0, scalar2=0.0,
                op0=mybir.AluOpType.add, op1=mybir.AluOpType.max,
            )
            nc.vector.tensor_scalar(
                out=t[:], in0=t[:], scalar1=6.0, scalar2=1.0 / 6.0,
                op0=mybir.AluOpType.min, op1=mybir.AluOpType.mult,
            )
            # h = s * t ; r = clip(h + noise, -0.5, 0.5)
            r = small.tile([128, NBT], FP32)
            nc.vector.tensor_tensor(
                out=r[:], in0=s_all[:], in1=t[:], op=mybir.AluOpType.mult,
            )
            nc.vector.tensor_tensor(
                out=r[:], in0=r[:], in1=noise_t[:], op=mybir.AluOpType.add,
            )
            nc.vector.tensor_scalar(
                out=r[:], in0=r[:], scalar1=-0.5, scalar2=0.5,
                op0=mybir.AluOpType.max, op1=mybir.AluOpType.min,
            )
            nc.gpsimd.dma_start(out_r, r[:])

    nc.compile()
    return nc


def _get_nc():
    if "nc" not in _CACHE:
        _CACHE["nc"] = _build()
    return _CACHE["nc"]


def kernel(x: np.ndarray, y: np.ndarray, noise: np.ndarray, **_run_kwargs) -> np.ndarray:
    x = np.ascontiguousarray(x, dtype=np.float32)
    y = np.ascontiguousarray(y, dtype=np.float32)
    noise = np.ascontiguousarray(noise, dtype=np.float32)

    nc = _get_nc()
    in_maps = [
        {
            "x": x[i * BL:(i + 1) * BL],
            "y": y[i * BL:(i + 1) * BL],
            "noise": noise[i * BL:(i + 1) * BL],
        }
        for i in range(NCORES)
    ]
    res = run_bass_kernel_spmd(nc, in_maps, list(range(NCORES)), **_run_kwargs)
    out = np.concatenate([res.results[i]["out"] for i in range(NCORES)], axis=0)
    if _run_kwargs:
        _CACHE["last_results"] = res
    return out
